# revision 17
# baseline (speedup 1.0000x reference)
"""Trainium2 Bass kernel: disentangled (DeBERTa-style) attention.

Full inputs in, full output out. Sharding: data-parallel over batch (4) x
tensor-parallel over head-groups (2) = 8 cores. Core c handles batch c//2,
heads (c%2)*6 .. +6. The relative-position tensors are replicated.

Key algebraic facts exploited:
  * P = table[rel] has only 513 distinct rows (rel depends on j-i only), so
    qr/kr = (P @ Wp) collapses to tableW = table @ Wp plus an index map.
  * c_p[i,s] = qc[i] . tableK[r],  c_r[i,s] = (tableQ[r] . kc_sum), with
    r = clip(i-s+256, 0, 512).  Both fold into one per-head strip
    CPc[i, r] = qc[i] . tableK[r] + cr[r]  of shape [S, 513].
  * The score contribution extra[i,s] = CPc[i, r(i,s)] is a Toeplitz skew of
    that strip: materialize a padded, reversed strip CPcE[i, u] (u in [0,768))
    in DRAM, then read 128x640 parallelogram tiles with a DMA access pattern
    whose partition stride is (768-1) elements -- each SBUF partition gets a
    contiguous run, so the DMA runs at line rate.  PE transpose-matmuls
    accumulate those tiles into the transposed score PSUM.  Fully saturated
    blocks (|i-s| > 383) are rank-1 and handled by K=65 matmuls whose
    stationary operand has satcol broadcast to every column.
  * Strip generation for pair p+1 is software-pipelined into the score loop
    of pair p (and pair 0's into phase A) so the PE instruction stream stays
    dense -- TRN2's HAM clock gate halves the PE clock whenever the stream
    goes idle-ish, which was the dominant cost of the phase-serial version.

Score layout is transposed ([s on partitions, i free]) so that attn@V needs
no transposes: out_raw[i,d] = sum_s exp[s,i] V[s,d] contracts s on the
partition dim, the softmax denominator rides along as a ones-column of V,
and normalization is a per-partition tensor_scalar.
"""

import math
from contextlib import ExitStack

import ml_dtypes
import numpy as np

import concourse.bass as bass
from concourse import bacc
import concourse.mybir as mybir
import concourse.tile as tile
from concourse.bass_utils import run_bass_kernel_spmd
from concourse.masks import make_identity

f32 = mybir.dt.float32
bf16 = mybir.dt.bfloat16

B, S, D = 4, 1024, 768
NH, DH, KC = 12, 64, 256
HPC = NH // 2          # heads per core = 6
DG = HPC * DH          # 384 head-dims per core
W_CPE = 768            # padded skew strip width (127 | 513 | 128)
NCORES = 8

LAST_RESULT = None     # BassKernelResults of the most recent run (for tests)


def _sat_ranges(J):
    """Fully saturated column ranges of transposed-score block-row J.

    Returns (sat_row, lo, hi) triples: sat_row 0 => r=512 (i-s >= 257),
    sat_row 1 => r=0 (i-s <= -257).  Ranges are split at the 512-column PSUM
    bank boundary.
    """
    out = []
    lo = 128 * (J + 3)           # i >= 128*(J+3)  -> r = 512
    if lo < S:
        for b0 in (0, 512):
            a, b = max(lo, b0), min(S, b0 + 512)
            if a < b:
                out.append((0, a, b))
    hi = 128 * (J - 2)           # i < 128*(J-2)   -> r = 0
    if hi > 0:
        for b0 in (0, 512):
            a, b = max(0, b0), min(hi, b0 + 512)
            if a < b:
                out.append((1, a, b))
    return out


def build_bass(with_bias=True):
    nc = bacc.Bacc("TRN2", target_bir_lowering=False)

    xtb = nc.dram_tensor("xtb", [D, S], bf16, kind="ExternalInput")
    wq = nc.dram_tensor("wq", [D, DG], bf16, kind="ExternalInput")
    wk = nc.dram_tensor("wk", [D, DG], bf16, kind="ExternalInput")
    wv = nc.dram_tensor("wv", [D, DG], bf16, kind="ExternalInput")
    bq = nc.dram_tensor("bq", [1, DG], bf16, kind="ExternalInput")
    bk = nc.dram_tensor("bk", [1, DG], bf16, kind="ExternalInput")
    bv = nc.dram_tensor("bv", [1, DG], bf16, kind="ExternalInput")
    cw = nc.dram_tensor("cw", [DG, D], bf16, kind="ExternalInput")
    tpad = nc.dram_tensor("tpad", [DH, W_CPE], bf16, kind="ExternalInput")
    wpq = nc.dram_tensor("wpq", [DH, DH], bf16, kind="ExternalInput")
    wpk = nc.dram_tensor("wpk", [DH, DH], bf16, kind="ExternalInput")
    mb = nc.dram_tensor("mb", [S], f32, kind="ExternalInput")
    out = nc.dram_tensor("out", [S, D], f32, kind="ExternalOutput")
    # per-pair skew strip scratch (separate tensors so pair p+1's writes
    # never serialize against pair p's reads)
    cpe = [nc.dram_tensor(f"cpe{t}", [2, S, W_CPE], bf16) for t in range(3)]

    with tile.TileContext(nc) as tc, ExitStack() as ex:
        const = ex.enter_context(tc.tile_pool(name="const", bufs=1))
        persist = ex.enter_context(tc.tile_pool(name="persist", bufs=1))
        # strip-generation pools live from phase A through the score loop
        psB = ex.enter_context(tc.tile_pool(name="psB", space="PSUM", bufs=2))
        cpool = ex.enter_context(tc.tile_pool(name="cpool", bufs=6))

        ident_b = const.tile([128, 128], bf16, name="ident_b")
        make_identity(nc, ident_b[:])
        ones_f = const.tile([1, 512], bf16, name="ones_f")
        nc.vector.memset(ones_f[:], 1.0)
        ones65 = const.tile([65, 128], bf16, name="ones65")
        nc.vector.memset(ones65[:], 1.0)
        mb_sb = const.tile([128, 8], f32, name="mb_sb")

        QT = [persist.tile([128, S], bf16, name=f"QT{t}") for t in range(3)]
        KT = [persist.tile([128, S], bf16, name=f"KT{t}") for t in range(3)]
        Vb = [persist.tile([128, HPC * 128], bf16, name=f"Vb{j}") for j in range(8)]
        cws = [persist.tile([128, D], bf16, name=f"cw{c}") for c in range(3)]
        TQp = persist.tile([DH, W_CPE], bf16, name="TQp")
        kcs = [persist.tile([128, 1], f32, name=f"kcs{t}") for t in range(3)]
        # satb[h][p]: [65,128] stationary whose every column = satcols[h][:,p]
        satb = [[persist.tile([65, 128], bf16, name=f"satb{h}_{p}") for p in range(2)]
                for h in range(HPC)]

        def copy_out(k, dst, src):
            """PSUM->SBUF drain; GPSIMD has no PSUM port, so alternate the
            two engines that do."""
            if k % 2 == 1:
                nc.scalar.copy(dst, src)
            else:
                nc.vector.tensor_copy(dst, src)

        # ---------------- Phase A: QKV projection + tables ----------------
        ab = ExitStack()
        wload = ab.enter_context(tc.tile_pool(name="wload", bufs=1))
        abp = ab.enter_context(tc.tile_pool(name="abp", bufs=1))

        xbs, wqs, wks, wvs = [], [], [], []
        for t in range(6):
            w = wload.tile([128, DG], bf16, name=f"wq{t}")
            nc.sync.dma_start(out=w[:], in_=wq[128 * t:128 * (t + 1), :])
            wqs.append(w)
            xb = wload.tile([128, S], bf16, name=f"xbt{t}")
            nc.sync.dma_start(out=xb[:], in_=xtb[128 * t:128 * (t + 1), :])
            xbs.append(xb)
        for nm, dram, lst in (("wk", wk, wks), ("wv", wv, wvs)):
            for t in range(6):
                w = wload.tile([128, DG], bf16, name=f"{nm}{t}")
                nc.sync.dma_start(out=w[:], in_=dram[128 * t:128 * (t + 1), :])
                lst.append(w)
        for c in range(3):
            nc.sync.dma_start(out=cws[c][:], in_=cw[128 * c:128 * (c + 1), :])
        bq_sb = wload.tile([1, DG], bf16, name="bq_sb")
        bk_sb = wload.tile([1, DG], bf16, name="bk_sb")
        bv_sb = wload.tile([1, DG], bf16, name="bv_sb")
        nc.sync.dma_start(out=bq_sb[:], in_=bq[:, :])
        nc.sync.dma_start(out=bk_sb[:], in_=bk[:, :])
        nc.sync.dma_start(out=bv_sb[:], in_=bv[:, :])
        nc.sync.dma_start(out=mb_sb[:], in_=bass.AP(mb, 0, [[1, 128], [128, 8]]))
        tpad_sb = wload.tile([DH, W_CPE], bf16, name="tpad_sb")
        nc.sync.dma_start(out=tpad_sb[:], in_=tpad[:, :])
        wpq_sb = wload.tile([DH, DH], bf16, name="wpq_sb")
        nc.sync.dma_start(out=wpq_sb[:], in_=wpq[:, :])
        wpk_sb = wload.tile([DH, DH], bf16, name="wpk_sb")
        nc.sync.dma_start(out=wpk_sb[:], in_=wpk[:, :])

        # per-head augmented operand tiles (all pairs; alive through scores)
        QTaug = [abp.tile([65, S], bf16, name=f"QTaug{h}") for h in range(HPC)]
        TKaug = [abp.tile([65, W_CPE], bf16, name=f"TKaug{h}") for h in range(HPC)]
        kc_col = [abp.tile([DH, 1], bf16, name=f"kc{h}") for h in range(HPC)]
        satcols = [abp.tile([65, 2], f32, name=f"satc{h}") for h in range(HPC)]
        crh_sb = [abp.tile([1, W_CPE], bf16, name=f"crh{h}") for h in range(HPC)]
        TKp_sb = wload.tile([DH, W_CPE], bf16, name="TKp_sb")

        strip_ct = {}        # (h, I) -> SBUF tile pending DMA (none kept)

        def emit_strip(h, I):
            """One 128-row block of head h's padded skew strip -> DRAM."""
            t, r = divmod(h, 2)
            lhs = QTaug[h][:, 128 * I:128 * (I + 1)]
            ct = cpool.tile([128, W_CPE], bf16, name="cpe_t")
            for k, (w0, w1) in enumerate(((0, 512), (512, W_CPE))):
                pab = psB.tile([128, w1 - w0], f32, name="psB_t", tag="psB")
                nc.tensor.matmul(pab[:], lhs, TKaug[h][:, w0:w1],
                                 start=True, stop=True)
                copy_out(2 * (8 * h + I) + k, ct[:, w0:w1], pab[:])
            nc.sync.dma_start(out=cpe[t][r, 128 * I:128 * (I + 1), :], in_=ct[:])

        def emit_qkt(dst, wlist, brow, m, n2):
            ps = psA.tile([128, 512], f32, name="psA_t", tag="psA")
            for kk in range(6):
                nc.tensor.matmul(
                    ps[:], wlist[kk][:, 128 * m:128 * (m + 1)],
                    xbs[kk][:, 512 * n2:512 * (n2 + 1)],
                    start=(kk == 0), stop=(kk == 5 and not with_bias))
            if with_bias:
                nc.tensor.matmul(
                    ps[:], brow[0:1, 128 * m:128 * (m + 1)],
                    ones_f[0:1, :], start=False, stop=True)
            copy_out(m + n2, dst[m][:, 512 * n2:512 * (n2 + 1)], ps[:])

        def emit_v(j):
            ps = psA.tile([128, DG], f32, name="psA_v", tag="psA")
            for kk in range(6):
                nc.tensor.matmul(
                    ps[:], xbs[kk][:, 128 * j:128 * (j + 1)], wvs[kk][:],
                    start=(kk == 0), stop=(kk == 5 and not with_bias))
            if with_bias:
                nc.tensor.matmul(ps[:], ones_f[0:1, 0:128], bv_sb[0:1, :],
                                 start=False, stop=True)
            vdst = Vb[j][:].rearrange("p (h c) -> p h c", h=HPC)
            nc.gpsimd.memset(vdst[:, :, 64:128], 0.0)
            nc.vector.tensor_copy(vdst[:, :, 0:64],
                                  ps[:].rearrange("p (h c) -> p h c", h=HPC))
            nc.gpsimd.memset(vdst[:, :, 64:65], 1.0)

        def emit_pair_setup(t):
            """kc_sum, QTaug/TKaug/crh/satcols/satb for pair t's two heads."""
            nc.vector.tensor_reduce(kcs[t][:], KT[t][:],
                                    axis=mybir.AxisListType.X,
                                    op=mybir.AluOpType.add)
            for r in range(2):
                h = 2 * t + r
                nc.sync.dma_start(out=QTaug[h][0:64, :],
                                  in_=QT[t][64 * r:64 * r + 64, :])
                nc.gpsimd.memset(QTaug[h][64:65, :], 1.0)
                nc.gpsimd.tensor_copy(TKaug[h][0:64, :], TKp_sb[:])
                nc.gpsimd.dma_start(out=kc_col[h][:],
                                    in_=kcs[t][64 * r:64 * r + 64, 0:1])
            for r in range(2):
                h = 2 * t + r
                ps = psT.tile([1, W_CPE], f32, name="psT_cr", tag="psT")
                nc.tensor.matmul(ps[:, 0:512], kc_col[h][:], TQp[:, 0:512],
                                 start=True, stop=True)
                nc.tensor.matmul(ps[:, 512:W_CPE], kc_col[h][:], TQp[:, 512:W_CPE],
                                 start=True, stop=True)
                nc.vector.tensor_copy(crh_sb[h][:], ps[:])
                nc.sync.dma_start(out=TKaug[h][64:65, :], in_=crh_sb[h][:])
                nc.gpsimd.tensor_copy(satcols[h][:, 0:1], TKaug[h][:, 127:128])
                nc.gpsimd.tensor_copy(satcols[h][:, 1:2], TKaug[h][:, 639:640])
                for p in range(2):
                    nc.gpsimd.tensor_scalar(
                        out=satb[h][p][:], in0=ones65[:],
                        scalar1=satcols[h][:, p:p + 1], scalar2=None,
                        op0=mybir.AluOpType.mult)

        with tc.tile_pool(name="psA", space="PSUM", bufs=3) as psA, \
             tc.tile_pool(name="psT", space="PSUM", bufs=1) as psT:
            # pair 0's Q^T/K^T first so its strip pipeline can start early
            for m, n2 in ((0, 0), (0, 1)):
                emit_qkt(QT, wqs, bq_sb, m, n2)
            for m, n2 in ((0, 0), (0, 1)):
                emit_qkt(KT, wks, bk_sb, m, n2)
            # tableW strips: TKp = Wp_k^T @ tpad, TQp = Wp_q^T @ tpad
            for wsb, dsts in ((wpk_sb, TKp_sb), (wpq_sb, TQp)):
                ps = psT.tile([DH, W_CPE], f32, name="psT_t", tag="psT")
                nc.tensor.matmul(ps[:, 0:512], wsb[:], tpad_sb[:, 0:512],
                                 start=True, stop=True)
                nc.tensor.matmul(ps[:, 512:W_CPE], wsb[:], tpad_sb[:, 512:W_CPE],
                                 start=True, stop=True)
                nc.vector.tensor_copy(dsts[:], ps[:])
            emit_pair_setup(0)

            # interleave pair 0's 16 strip blocks with the rest of phase A
            rest = [("qkt", QT, wqs, bq_sb, 1, 0), ("qkt", QT, wqs, bq_sb, 1, 1),
                    ("qkt", KT, wks, bk_sb, 1, 0), ("qkt", KT, wks, bk_sb, 1, 1),
                    ("setup", 1),
                    ("qkt", QT, wqs, bq_sb, 2, 0), ("qkt", QT, wqs, bq_sb, 2, 1),
                    ("qkt", KT, wks, bk_sb, 2, 0), ("qkt", KT, wks, bk_sb, 2, 1),
                    ("setup", 2),
                    ("v", 0), ("v", 1), ("v", 2), ("v", 3),
                    ("v", 4), ("v", 5), ("v", 6), ("v", 7)]
            strips0 = [(h, I) for I in range(8) for h in (0, 1)]
            ri = si = 0
            while ri < len(rest) or si < len(strips0):
                if ri < len(rest):
                    op_ = rest[ri]; ri += 1
                    if op_[0] == "qkt":
                        emit_qkt(*op_[1:])
                    elif op_[0] == "v":
                        emit_v(op_[1])
                    else:
                        emit_pair_setup(op_[1])
                if si < len(strips0):
                    h, I = strips0[si]; si += 1
                    emit_strip(h, I)

        # ----- score/attn loop per head-pair, strips(p+1) pipelined in -----
        hoT = [persist.tile([128, S], bf16, name=f"hoT{c}") for c in range(3)]
        with tc.tile_pool(name="psC", space="PSUM", bufs=2) as psC, \
             tc.tile_pool(name="psD", space="PSUM", bufs=2) as psD, \
             tc.tile_pool(name="srp", bufs=18) as srp, \
             tc.tile_pool(name="expp", bufs=16) as expp, \
             tc.tile_pool(name="rcp", bufs=4) as rcp, \
             tc.tile_pool(name="rbp", bufs=2) as rbp, \
             tc.tile_pool(name="outp", bufs=3) as outp, \
             tc.tile_pool(name="otp", bufs=2) as otp:

            def emit_d_group(dstate):
                """Emit one attn@V (h, half) group of the previous pair."""
                h, half, exps = dstate.pop(0)
                t, r = divmod(h, 2)
                pd = psD.tile([128, 512], f32, name="pd", tag="pdc")
                for J in range(8):
                    nc.tensor.matmul(
                        pd[:], Vb[J][:, 128 * h:128 * (h + 1)],
                        exps[(h, J)][:, 512 * half:512 * (half + 1)],
                        start=(J == 0), stop=(J == 7))
                rc = rcp.tile([1, 512], f32, name="rc")
                nc.vector.reciprocal(rc[:], pd[64:65, :])
                rb = rbp.tile([64, 512], f32, name="rb")
                nc.gpsimd.partition_broadcast(rb[:], rc[:])
                if not r:
                    dstv = hoT[t][0:64, 512 * half:512 * (half + 1)]
                else:
                    tmpo = dstate_tmp.setdefault(h, otp.tile([64, S], bf16, name="tmpo"))
                    dstv = tmpo[:, 512 * half:512 * (half + 1)]
                nc.vector.tensor_tensor(out=dstv, in0=pd[0:64, :], in1=rb[:],
                                        op=mybir.AluOpType.mult)
                if r:
                    tm = dstate_tmp[h]
                    nc.sync.dma_start(out=hoT[t][64:128, 512 * half:512 * (half + 1)],
                                      in_=tm[:, 512 * half:512 * (half + 1)])
                    if half == 1:
                        dstate_tmp.pop(h)

            pending = []          # (h, half, expT-dict) groups awaiting attn@V
            dstate_tmp = {}
            prefetched = {}       # (h, I) -> sr tile loaded ahead of its pair

            def load_sr(h, I):
                t, r = divmod(h, 2)
                sr = srp.tile([128, 640], bf16, name="sr")
                s_lo = max(0, 128 * (I - 2))
                s_hi = min(S, 128 * (I + 3))
                s0 = s_lo - 128 * (I - 2)
                Wd = s_hi - s_lo
                off = (r * S + 128 * I) * W_CPE + 127 + s0
                srcap = bass.AP(cpe[t], off, [[W_CPE - 1, 128], [1, Wd]])
                nc.sync.dma_start(out=sr[:, s0:s0 + Wd], in_=srcap)
                return sr

            for hp in range(3):
                heads = (2 * hp, 2 * hp + 1)
                SRs = {h: {} for h in heads}
                expT = {}
                for J in range(8):
                    # next pair's strip blocks, two per step
                    if hp < 2:
                        for h2 in (2 * hp + 2, 2 * hp + 3):
                            emit_strip(h2, J)
                    # prefetch distance 2: tile I is issued at step I-4 so the
                    # ~2.5us DMA (latency + 160KB) never gates the transposes
                    new_Is = range(0, 5) if J == 0 else \
                        (range(J + 4, J + 5) if J + 4 < 8 else range(0))
                    for h in heads:
                        for I in new_Is:
                            if (h, I) in prefetched:
                                SRs[h][I] = prefetched.pop((h, I))
                            else:
                                SRs[h][I] = load_sr(h, I)
                    if hp < 2 and 4 <= J <= 7:
                        # warm up next pair's J=0 window (2 tiles per step)
                        I2 = J - 4
                        for h2 in (2 * hp + 2, 2 * hp + 3):
                            prefetched[(h2, I2)] = load_sr(h2, I2)
                    scs = {}
                    for h in heads:
                        bank_ops = {0: [("cc", 0)], 1: [("cc", 1)]}
                        for I in range(max(0, J - 2), min(8, J + 3)):
                            bank_ops[I // 4].append(("tr", I))
                        for row, lo, hi in _sat_ranges(J):
                            bank_ops[lo // 512].append(("sat", (row, lo, hi)))
                        last = {b: ops[-1] for b, ops in bank_ops.items()}
                        sc = psC.tile([128, S], f32, name="sc")
                        scs[h] = (sc, [sc[:, 0:512], sc[:, 512:S]], last)
                    for n2 in range(2):
                        for h in heads:
                            t, r = divmod(h, 2)
                            sc, halves, last = scs[h]
                            nc.tensor.matmul(
                                halves[n2],
                                KT[t][64 * r:64 * r + 64, 128 * J:128 * (J + 1)],
                                QT[t][64 * r:64 * r + 64, 512 * n2:512 * (n2 + 1)],
                                start=True, stop=(last[n2] == ("cc", n2)),
                                tile_position=(64 * r, 0) if r else None)
                    for h in heads:
                        sc, halves, last = scs[h]
                        for I in range(max(0, J - 2), min(8, J + 3)):
                            dlt = I - J
                            nc.tensor.matmul(
                                halves[I // 4][:, 128 * (I % 4):128 * (I % 4 + 1)],
                                SRs[h][I][:, 128 * (2 - dlt):128 * (3 - dlt)],
                                ident_b[:],
                                start=False,
                                stop=(last[I // 4] == ("tr", I)))
                        for row, lo, hi in _sat_ranges(J):
                            b = lo // 512
                            nc.tensor.matmul(
                                halves[b][:, lo - 512 * b:hi - 512 * b],
                                satb[h][row][:], QTaug[h][:, lo:hi],
                                start=False,
                                stop=(last[b] == ("sat", (row, lo, hi))))
                        et = expp.tile([128, S], bf16, name="et")
                        nc.scalar.activation(et[:], sc[:],
                                             mybir.ActivationFunctionType.Exp,
                                             bias=mb_sb[:, J:J + 1], scale=1.0)
                        expT[(h, J)] = et
                    # interleave: drain ~1-2 previous-pair attn@V groups per J
                    for _ in range(2):
                        if pending:
                            emit_d_group(pending)
                for h in heads:
                    for half in range(2):
                        pending.append((h, half, expT))
            pending.sort(key=lambda g: g[1])

            # ---- Phase E (c_proj) interleaved with the final drains ----
            def emit_e(ic):
                ot = outp.tile([128, D], f32, name="ot")
                for n2 in range(2):
                    pc = psD.tile([128, 384], f32, name="pc", tag="pdc")
                    for c in range(3):
                        nc.tensor.matmul(pc[:], hoT[c][:, 128 * ic:128 * (ic + 1)],
                                         cws[c][:, 384 * n2:384 * (n2 + 1)],
                                         start=(c == 0), stop=(c == 2))
                    copy_out(2 * ic + n2, ot[:, 384 * n2:384 * (n2 + 1)], pc[:])
                nc.sync.dma_start(out=out[128 * ic:128 * (ic + 1), :], in_=ot[:])

            emit_d_group(pending)          # (h4, half 0)
            emit_d_group(pending)          # (h5, half 0) -> half 0 complete
            emit_e(0)
            emit_d_group(pending)          # (h4, half 1)
            emit_e(1)
            emit_d_group(pending)          # (h5, half 1)
            for ic in range(2, 8):
                emit_e(ic)
        ab.close()  # frees x/W/QTaug/TKaug sbuf

    nc.compile()
    return nc


_NC_CACHE = None
_NC_KEY = None


def _get_nc(with_bias=True):
    global _NC_CACHE, _NC_KEY
    if _NC_CACHE is None or _NC_KEY != with_bias:
        _NC_CACHE = build_bass(with_bias=with_bias)
        _NC_KEY = with_bias
    return _NC_CACHE


def make_in_maps(x, attention_mask, Wc_w, Wc_b, Wp_w, table, cproj_w):
    x = np.asarray(x, np.float32)
    attention_mask = np.asarray(attention_mask)
    Wc_w = np.asarray(Wc_w, np.float32)
    Wc_b = np.asarray(Wc_b, np.float32)
    Wp_w = np.asarray(Wp_w, np.float32)
    table = np.asarray(table, np.float32)
    cproj_w = np.asarray(cproj_w, np.float32)

    scale = 1.0 / math.sqrt(DH)
    idx = np.clip(639 - np.arange(W_CPE), 0, 512)
    tpad_np = np.ascontiguousarray(table.T[:, idx])
    wpq_np = np.ascontiguousarray(Wp_w[:, 0:DH]) * scale
    wpk_np = np.ascontiguousarray(Wp_w[:, DH:2 * DH])

    in_maps = []
    for c in range(NCORES):
        b, hg = divmod(c, 2)
        sl = slice(hg * DG, (hg + 1) * DG)
        bf = ml_dtypes.bfloat16
        xt_c = np.ascontiguousarray(x[b].T)
        in_maps.append({
            "xtb": xt_c.astype(bf),
            "wq": (np.ascontiguousarray(Wc_w[:, sl]) * scale).astype(bf),
            "wk": np.ascontiguousarray(Wc_w[:, D + hg * DG: D + (hg + 1) * DG]).astype(bf),
            "wv": np.ascontiguousarray(Wc_w[:, 2 * D + hg * DG: 2 * D + (hg + 1) * DG]).astype(bf),
            "bq": (Wc_b[sl] * scale).reshape(1, DG).astype(bf),
            "bk": Wc_b[D + hg * DG: D + (hg + 1) * DG].reshape(1, DG).astype(bf),
            "bv": Wc_b[2 * D + hg * DG: 2 * D + (hg + 1) * DG].reshape(1, DG).astype(bf),
            "cw": np.ascontiguousarray(cproj_w[sl, :]).astype(bf),
            "tpad": tpad_np.astype(bf),
            "wpq": wpq_np.astype(bf),
            "wpk": wpk_np.astype(bf),
            "mb": np.where(attention_mask[b] == 0, -1e9, 0.0).astype(np.float32),
        })
    return in_maps


def kernel(x, attention_mask, Wc_w, Wc_b, Wp_w, table, cproj_w, cproj_b,
           n_h, k, **_ignored):
    global LAST_RESULT
    assert int(n_h) == NH and int(k) == KC
    in_maps = make_in_maps(x, attention_mask, Wc_w, Wc_b, Wp_w, table, cproj_w)
    wb = bool(np.any(np.asarray(Wc_b) != 0))
    nc = _get_nc(with_bias=wb)
    res = run_bass_kernel_spmd(nc, in_maps, list(range(NCORES)))
    LAST_RESULT = res
    outs = res.results
    full = np.zeros((B, S, D), np.float32)
    for b in range(B):
        full[b] = outs[2 * b]["out"] + outs[2 * b + 1]["out"]
    full += np.asarray(cproj_b, np.float32)[None, None, :]
    return full


# revision 18
# speedup vs baseline: 1.3011x; 1.3011x over previous
"""Trainium2 Bass kernel: disentangled (DeBERTa-style) attention.

Full inputs in, full output out. Sharding: data-parallel over batch (4) x
tensor-parallel over head-groups (2) = 8 cores. Core c handles batch c//2,
heads (c%2)*6 .. +6. The relative-position tensors are replicated.

Key algebraic facts exploited:
  * P = table[rel] has only 513 distinct rows (rel depends on j-i only), so
    qr/kr = (P @ Wp) collapses to tableW = table @ Wp plus an index map.
  * c_p[i,s] = qc[i] . tableK[r],  c_r[i,s] = (tableQ[r] . kc_sum), with
    r = clip(i-s+256, 0, 512).  Both fold into one per-head strip
    CPc[i, r] = qc[i] . tableK[r] + cr[r]  of shape [S, 513].
  * The score contribution extra[i,s] = CPc[i, r(i,s)] is a Toeplitz skew of
    that strip: materialize a padded, reversed strip CPcE[i, u] (u in [0,768))
    in DRAM, then read 128x640 parallelogram tiles with a DMA access pattern
    whose partition stride is (768-1) elements -- each SBUF partition gets a
    contiguous run, so the DMA runs at line rate.  PE transpose-matmuls
    accumulate those tiles into the transposed score PSUM.  Fully saturated
    blocks (|i-s| > 383) are rank-1 and handled by K=65 matmuls whose
    stationary operand has satcol broadcast to every column.
  * Strip generation for pair p+1 is software-pipelined into the score loop
    of pair p (and pair 0's into phase A) so the PE instruction stream stays
    dense -- TRN2's HAM clock gate halves the PE clock whenever the stream
    goes idle-ish, which was the dominant cost of the phase-serial version.

Score layout is transposed ([s on partitions, i free]) so that attn@V needs
no transposes: out_raw[i,d] = sum_s exp[s,i] V[s,d] contracts s on the
partition dim, the softmax denominator rides along as a ones-column of V,
and normalization is a per-partition tensor_scalar.
"""

import math
from contextlib import ExitStack

import ml_dtypes
import numpy as np

import concourse.bass as bass
from concourse import bacc
import concourse.mybir as mybir
import concourse.tile as tile
from concourse.bass_utils import run_bass_kernel_spmd
from concourse.masks import make_identity

f32 = mybir.dt.float32
bf16 = mybir.dt.bfloat16

B, S, D = 4, 1024, 768
NH, DH, KC = 12, 64, 256
HPC = NH // 2          # heads per core = 6
DG = HPC * DH          # 384 head-dims per core
W_CPE = 768            # padded skew strip width (127 | 513 | 128)
NCORES = 8

LAST_RESULT = None     # BassKernelResults of the most recent run (for tests)


def _sat_ranges(J):
    """Fully saturated column ranges of transposed-score block-row J.

    Returns (sat_row, lo, hi) triples: sat_row 0 => r=512 (i-s >= 257),
    sat_row 1 => r=0 (i-s <= -257).  Ranges are split at the 512-column PSUM
    bank boundary.
    """
    out = []
    lo = 128 * (J + 3)           # i >= 128*(J+3)  -> r = 512
    if lo < S:
        for b0 in (0, 512):
            a, b = max(lo, b0), min(S, b0 + 512)
            if a < b:
                out.append((0, a, b))
    hi = 128 * (J - 2)           # i < 128*(J-2)   -> r = 0
    if hi > 0:
        for b0 in (0, 512):
            a, b = max(0, b0), min(hi, b0 + 512)
            if a < b:
                out.append((1, a, b))
    return out


def build_bass(with_bias=True):
    nc = bacc.Bacc("TRN2", target_bir_lowering=False)

    xtb = nc.dram_tensor("xtb", [D, S], bf16, kind="ExternalInput")
    wq = nc.dram_tensor("wq", [D, DG], bf16, kind="ExternalInput")
    wk = nc.dram_tensor("wk", [D, DG], bf16, kind="ExternalInput")
    wv = nc.dram_tensor("wv", [D, DG], bf16, kind="ExternalInput")
    bq = nc.dram_tensor("bq", [1, DG], bf16, kind="ExternalInput")
    bk = nc.dram_tensor("bk", [1, DG], bf16, kind="ExternalInput")
    bv = nc.dram_tensor("bv", [1, DG], bf16, kind="ExternalInput")
    cw = nc.dram_tensor("cw", [DG, D], bf16, kind="ExternalInput")
    tpad = nc.dram_tensor("tpad", [DH, W_CPE], bf16, kind="ExternalInput")
    wpq = nc.dram_tensor("wpq", [DH, DH], bf16, kind="ExternalInput")
    wpk = nc.dram_tensor("wpk", [DH, DH], bf16, kind="ExternalInput")
    mb = nc.dram_tensor("mb", [S], f32, kind="ExternalInput")
    out = nc.dram_tensor("out", [S, D], f32, kind="ExternalOutput")
    # per-pair skew strip scratch (separate tensors so pair p+1's writes
    # never serialize against pair p's reads)
    cpe = [nc.dram_tensor(f"cpe{t}", [2, S, W_CPE], bf16) for t in range(3)]

    with tile.TileContext(nc) as tc, ExitStack() as ex:
        const = ex.enter_context(tc.tile_pool(name="const", bufs=1))
        persist = ex.enter_context(tc.tile_pool(name="persist", bufs=1))
        # strip-generation pools live from phase A through the score loop
        psB = ex.enter_context(tc.tile_pool(name="psB", space="PSUM", bufs=2))
        cpool = ex.enter_context(tc.tile_pool(name="cpool", bufs=6))

        ident_b = const.tile([128, 128], bf16, name="ident_b")
        make_identity(nc, ident_b[:])
        ones_f = const.tile([1, 512], bf16, name="ones_f")
        nc.vector.memset(ones_f[:], 1.0)
        ones65 = const.tile([65, 128], bf16, name="ones65")
        nc.vector.memset(ones65[:], 1.0)
        mb_sb = const.tile([128, 8], f32, name="mb_sb")

        QT = [persist.tile([128, S], bf16, name=f"QT{t}") for t in range(3)]
        KT = [persist.tile([128, S], bf16, name=f"KT{t}") for t in range(3)]
        Vb = [persist.tile([128, HPC * 128], bf16, name=f"Vb{j}") for j in range(8)]
        cws = [persist.tile([128, D], bf16, name=f"cw{c}") for c in range(3)]
        TQp = persist.tile([DH, W_CPE], bf16, name="TQp")
        kcs = [persist.tile([128, 1], f32, name=f"kcs{t}") for t in range(3)]
        # satb[h][p]: [65,128] stationary whose every column = satcols[h][:,p]
        satb = [[persist.tile([65, 128], bf16, name=f"satb{h}_{p}") for p in range(2)]
                for h in range(HPC)]

        def copy_out(k, dst, src):
            """PSUM->SBUF drain; GPSIMD has no PSUM port, so alternate the
            two engines that do."""
            if k % 2 == 1:
                nc.scalar.copy(dst, src)
            else:
                nc.vector.tensor_copy(dst, src)

        # ---------------- Phase A: QKV projection + tables ----------------
        ab = ExitStack()
        wload = ab.enter_context(tc.tile_pool(name="wload", bufs=1))
        abp = ab.enter_context(tc.tile_pool(name="abp", bufs=1))

        xbs, wqs, wks, wvs = [], [], [], []
        for t in range(6):
            w = wload.tile([128, DG], bf16, name=f"wq{t}")
            nc.sync.dma_start(out=w[:], in_=wq[128 * t:128 * (t + 1), :])
            wqs.append(w)
            xb = wload.tile([128, S], bf16, name=f"xbt{t}")
            nc.sync.dma_start(out=xb[:], in_=xtb[128 * t:128 * (t + 1), :])
            xbs.append(xb)
        for nm, dram, lst in (("wk", wk, wks), ("wv", wv, wvs)):
            for t in range(6):
                w = wload.tile([128, DG], bf16, name=f"{nm}{t}")
                nc.sync.dma_start(out=w[:], in_=dram[128 * t:128 * (t + 1), :])
                lst.append(w)
        for c in range(3):
            nc.sync.dma_start(out=cws[c][:], in_=cw[128 * c:128 * (c + 1), :])
        bq_sb = wload.tile([1, DG], bf16, name="bq_sb")
        bk_sb = wload.tile([1, DG], bf16, name="bk_sb")
        bv_sb = wload.tile([1, DG], bf16, name="bv_sb")
        nc.sync.dma_start(out=bq_sb[:], in_=bq[:, :])
        nc.sync.dma_start(out=bk_sb[:], in_=bk[:, :])
        nc.sync.dma_start(out=bv_sb[:], in_=bv[:, :])
        nc.sync.dma_start(out=mb_sb[:], in_=bass.AP(mb, 0, [[1, 128], [128, 8]]))
        tpad_sb = wload.tile([DH, W_CPE], bf16, name="tpad_sb")
        nc.sync.dma_start(out=tpad_sb[:], in_=tpad[:, :])
        wpq_sb = wload.tile([DH, DH], bf16, name="wpq_sb")
        nc.sync.dma_start(out=wpq_sb[:], in_=wpq[:, :])
        wpk_sb = wload.tile([DH, DH], bf16, name="wpk_sb")
        nc.sync.dma_start(out=wpk_sb[:], in_=wpk[:, :])

        # per-head augmented operand tiles (all pairs; alive through scores)
        QTaug = [abp.tile([65, S], bf16, name=f"QTaug{h}") for h in range(HPC)]
        TKaug = [abp.tile([65, W_CPE], bf16, name=f"TKaug{h}") for h in range(HPC)]
        kc_col = [abp.tile([DH, 1], bf16, name=f"kc{h}") for h in range(HPC)]
        satcols = [abp.tile([65, 2], f32, name=f"satc{h}") for h in range(HPC)]
        crh_sb = [abp.tile([1, W_CPE], bf16, name=f"crh{h}") for h in range(HPC)]
        TKp_sb = wload.tile([DH, W_CPE], bf16, name="TKp_sb")

        strip_ct = {}        # (h, I) -> SBUF tile pending DMA (none kept)

        def emit_strip(h, I):
            """One 128-row block of head h's padded skew strip -> DRAM."""
            t, r = divmod(h, 2)
            lhs = QTaug[h][:, 128 * I:128 * (I + 1)]
            ct = cpool.tile([128, W_CPE], bf16, name="cpe_t")
            for k, (w0, w1) in enumerate(((0, 512), (512, W_CPE))):
                pab = psB.tile([128, w1 - w0], f32, name="psB_t", tag="psB")
                nc.tensor.matmul(pab[:], lhs, TKaug[h][:, w0:w1],
                                 start=True, stop=True)
                copy_out(2 * (8 * h + I) + k, ct[:, w0:w1], pab[:])
            nc.sync.dma_start(out=cpe[t][r, 128 * I:128 * (I + 1), :], in_=ct[:])

        def emit_qkt(dst, wlist, brow, m, n2):
            ps = psA.tile([128, 512], f32, name="psA_t", tag="psA")
            for kk in range(6):
                nc.tensor.matmul(
                    ps[:], wlist[kk][:, 128 * m:128 * (m + 1)],
                    xbs[kk][:, 512 * n2:512 * (n2 + 1)],
                    start=(kk == 0), stop=(kk == 5 and not with_bias))
            if with_bias:
                nc.tensor.matmul(
                    ps[:], brow[0:1, 128 * m:128 * (m + 1)],
                    ones_f[0:1, :], start=False, stop=True)
            copy_out(m + n2, dst[m][:, 512 * n2:512 * (n2 + 1)], ps[:])

        def emit_v(j):
            ps = psA.tile([128, DG], f32, name="psA_v", tag="psA")
            for kk in range(6):
                nc.tensor.matmul(
                    ps[:], xbs[kk][:, 128 * j:128 * (j + 1)], wvs[kk][:],
                    start=(kk == 0), stop=(kk == 5 and not with_bias))
            if with_bias:
                nc.tensor.matmul(ps[:], ones_f[0:1, 0:128], bv_sb[0:1, :],
                                 start=False, stop=True)
            vdst = Vb[j][:].rearrange("p (h c) -> p h c", h=HPC)
            nc.vector.memset(vdst[:, :, 64:128], 0.0)
            nc.vector.tensor_copy(vdst[:, :, 0:64],
                                  ps[:].rearrange("p (h c) -> p h c", h=HPC))
            nc.vector.memset(vdst[:, :, 64:65], 1.0)

        def emit_pair_setup(t):
            """kc_sum, QTaug/TKaug/crh/satcols/satb for pair t's two heads."""
            nc.vector.tensor_reduce(kcs[t][:], KT[t][:],
                                    axis=mybir.AxisListType.X,
                                    op=mybir.AluOpType.add)
            for r in range(2):
                h = 2 * t + r
                nc.sync.dma_start(out=QTaug[h][0:64, :],
                                  in_=QT[t][64 * r:64 * r + 64, :])
                nc.vector.memset(QTaug[h][64:65, :], 1.0)
                nc.vector.tensor_copy(TKaug[h][0:64, :], TKp_sb[:])
                nc.gpsimd.dma_start(out=kc_col[h][:],
                                    in_=kcs[t][64 * r:64 * r + 64, 0:1])
            for r in range(2):
                h = 2 * t + r
                ps = psT.tile([1, W_CPE], f32, name="psT_cr", tag="psT")
                nc.tensor.matmul(ps[:, 0:512], kc_col[h][:], TQp[:, 0:512],
                                 start=True, stop=True)
                nc.tensor.matmul(ps[:, 512:W_CPE], kc_col[h][:], TQp[:, 512:W_CPE],
                                 start=True, stop=True)
                nc.vector.tensor_copy(crh_sb[h][:], ps[:])
                nc.sync.dma_start(out=TKaug[h][64:65, :], in_=crh_sb[h][:])
                nc.vector.tensor_copy(satcols[h][:, 0:1], TKaug[h][:, 127:128])
                nc.vector.tensor_copy(satcols[h][:, 1:2], TKaug[h][:, 639:640])
                for p in range(2):
                    nc.vector.tensor_scalar(
                        out=satb[h][p][:], in0=ones65[:],
                        scalar1=satcols[h][:, p:p + 1], scalar2=None,
                        op0=mybir.AluOpType.mult)

        with tc.tile_pool(name="psA", space="PSUM", bufs=2) as psA, \
             tc.tile_pool(name="psT", space="PSUM", bufs=2) as psT:
            # pair 0's Q^T/K^T first so its strip pipeline can start early
            for m, n2 in ((0, 0), (0, 1)):
                emit_qkt(QT, wqs, bq_sb, m, n2)
            for m, n2 in ((0, 0), (0, 1)):
                emit_qkt(KT, wks, bk_sb, m, n2)
            # tableW strips: TKp = Wp_k^T @ tpad, TQp = Wp_q^T @ tpad
            for wsb, dsts in ((wpk_sb, TKp_sb), (wpq_sb, TQp)):
                ps = psT.tile([DH, W_CPE], f32, name="psT_t", tag="psT")
                nc.tensor.matmul(ps[:, 0:512], wsb[:], tpad_sb[:, 0:512],
                                 start=True, stop=True)
                nc.tensor.matmul(ps[:, 512:W_CPE], wsb[:], tpad_sb[:, 512:W_CPE],
                                 start=True, stop=True)
                nc.vector.tensor_copy(dsts[:], ps[:])
            emit_pair_setup(0)

            # interleave pair 0's 16 strip blocks with the rest of phase A
            rest = [("qkt", QT, wqs, bq_sb, 1, 0), ("qkt", QT, wqs, bq_sb, 1, 1),
                    ("qkt", KT, wks, bk_sb, 1, 0), ("qkt", KT, wks, bk_sb, 1, 1),
                    ("setup", 1),
                    ("qkt", QT, wqs, bq_sb, 2, 0), ("qkt", QT, wqs, bq_sb, 2, 1),
                    ("qkt", KT, wks, bk_sb, 2, 0), ("qkt", KT, wks, bk_sb, 2, 1),
                    ("setup", 2),
                    ("v", 0), ("v", 1), ("v", 2), ("v", 3),
                    ("v", 4), ("v", 5), ("v", 6), ("v", 7)]
            strips0 = [(h, I) for I in range(8) for h in (0, 1)]
            ri = si = 0
            while ri < len(rest) or si < len(strips0):
                if ri < len(rest):
                    op_ = rest[ri]; ri += 1
                    if op_[0] == "qkt":
                        emit_qkt(*op_[1:])
                    elif op_[0] == "v":
                        emit_v(op_[1])
                    else:
                        emit_pair_setup(op_[1])
                if si < len(strips0):
                    h, I = strips0[si]; si += 1
                    emit_strip(h, I)

        # ----- score/attn loop per head-pair, strips(p+1) pipelined in -----
        hoT = [persist.tile([128, S], bf16, name=f"hoT{c}") for c in range(3)]
        with tc.tile_pool(name="psC", space="PSUM", bufs=2) as psC, \
             tc.tile_pool(name="psD", space="PSUM", bufs=2) as psD, \
             tc.tile_pool(name="srp", bufs=18) as srp, \
             tc.tile_pool(name="expp", bufs=16) as expp, \
             tc.tile_pool(name="rcp", bufs=4) as rcp, \
             tc.tile_pool(name="rbp", bufs=2) as rbp, \
             tc.tile_pool(name="outp", bufs=3) as outp, \
             tc.tile_pool(name="otp", bufs=2) as otp:

            def emit_d_group(dstate):
                """Emit one attn@V (h, half) group of the previous pair."""
                h, half, exps = dstate.pop(0)
                t, r = divmod(h, 2)
                pd = psD.tile([128, 512], f32, name="pd", tag="pdc")
                for J in range(8):
                    nc.tensor.matmul(
                        pd[:], Vb[J][:, 128 * h:128 * (h + 1)],
                        exps[(h, J)][:, 512 * half:512 * (half + 1)],
                        start=(J == 0), stop=(J == 7))
                rc = rcp.tile([1, 512], f32, name="rc")
                nc.vector.reciprocal(rc[:], pd[64:65, :])
                rb = rbp.tile([64, 512], f32, name="rb")
                nc.gpsimd.partition_broadcast(rb[:], rc[:])
                if not r:
                    dstv = hoT[t][0:64, 512 * half:512 * (half + 1)]
                else:
                    tmpo = dstate_tmp.setdefault(h, otp.tile([64, S], bf16, name="tmpo"))
                    dstv = tmpo[:, 512 * half:512 * (half + 1)]
                nc.vector.tensor_tensor(out=dstv, in0=pd[0:64, :], in1=rb[:],
                                        op=mybir.AluOpType.mult)
                if r:
                    tm = dstate_tmp[h]
                    nc.sync.dma_start(out=hoT[t][64:128, 512 * half:512 * (half + 1)],
                                      in_=tm[:, 512 * half:512 * (half + 1)])
                    if half == 1:
                        dstate_tmp.pop(h)

            pending = []          # (h, half, expT-dict) groups awaiting attn@V
            dstate_tmp = {}
            prefetched = {}       # (h, I) -> sr tile loaded ahead of its pair

            def load_sr(h, I):
                t, r = divmod(h, 2)
                sr = srp.tile([128, 640], bf16, name="sr")
                s_lo = max(0, 128 * (I - 2))
                s_hi = min(S, 128 * (I + 3))
                s0 = s_lo - 128 * (I - 2)
                Wd = s_hi - s_lo
                off = (r * S + 128 * I) * W_CPE + 127 + s0
                srcap = bass.AP(cpe[t], off, [[W_CPE - 1, 128], [1, Wd]])
                nc.sync.dma_start(out=sr[:, s0:s0 + Wd], in_=srcap)
                return sr

            for hp in range(3):
                heads = (2 * hp, 2 * hp + 1)
                SRs = {h: {} for h in heads}
                expT = {}
                for J in range(8):
                    # next pair's strip blocks, two per step
                    if hp < 2:
                        for h2 in (2 * hp + 2, 2 * hp + 3):
                            emit_strip(h2, J)
                    # prefetch distance 2: tile I is issued at step I-4 so the
                    # ~2.5us DMA (latency + 160KB) never gates the transposes
                    new_Is = range(0, 5) if J == 0 else \
                        (range(J + 4, J + 5) if J + 4 < 8 else range(0))
                    for h in heads:
                        for I in new_Is:
                            if (h, I) in prefetched:
                                SRs[h][I] = prefetched.pop((h, I))
                            else:
                                SRs[h][I] = load_sr(h, I)
                    if hp < 2 and 4 <= J <= 7:
                        # warm up next pair's J=0 window (2 tiles per step)
                        I2 = J - 4
                        for h2 in (2 * hp + 2, 2 * hp + 3):
                            prefetched[(h2, I2)] = load_sr(h2, I2)
                    scs = {}
                    for h in heads:
                        bank_ops = {0: [("cc", 0)], 1: [("cc", 1)]}
                        for I in range(max(0, J - 2), min(8, J + 3)):
                            bank_ops[I // 4].append(("tr", I))
                        for row, lo, hi in _sat_ranges(J):
                            bank_ops[lo // 512].append(("sat", (row, lo, hi)))
                        last = {b: ops[-1] for b, ops in bank_ops.items()}
                        sc = psC.tile([128, S], f32, name="sc")
                        scs[h] = (sc, [sc[:, 0:512], sc[:, 512:S]], last)
                    for n2 in range(2):
                        for h in heads:
                            t, r = divmod(h, 2)
                            sc, halves, last = scs[h]
                            nc.tensor.matmul(
                                halves[n2],
                                KT[t][64 * r:64 * r + 64, 128 * J:128 * (J + 1)],
                                QT[t][64 * r:64 * r + 64, 512 * n2:512 * (n2 + 1)],
                                start=True, stop=(last[n2] == ("cc", n2)),
                                tile_position=(64 * r, 0) if r else None)
                    for h in heads:
                        sc, halves, last = scs[h]
                        for I in range(max(0, J - 2), min(8, J + 3)):
                            dlt = I - J
                            nc.tensor.matmul(
                                halves[I // 4][:, 128 * (I % 4):128 * (I % 4 + 1)],
                                SRs[h][I][:, 128 * (2 - dlt):128 * (3 - dlt)],
                                ident_b[:],
                                start=False,
                                stop=(last[I // 4] == ("tr", I)))
                        for row, lo, hi in _sat_ranges(J):
                            b = lo // 512
                            nc.tensor.matmul(
                                halves[b][:, lo - 512 * b:hi - 512 * b],
                                satb[h][row][:], QTaug[h][:, lo:hi],
                                start=False,
                                stop=(last[b] == ("sat", (row, lo, hi))))
                        et = expp.tile([128, S], bf16, name="et")
                        nc.scalar.activation(et[:], sc[:],
                                             mybir.ActivationFunctionType.Exp,
                                             bias=mb_sb[:, J:J + 1], scale=1.0)
                        expT[(h, J)] = et
                    # interleave: drain ~1-2 previous-pair attn@V groups per J
                    for _ in range(2):
                        if pending:
                            emit_d_group(pending)
                for h in heads:
                    for half in range(2):
                        pending.append((h, half, expT))
            pending.sort(key=lambda g: g[1])

            # ---- Phase E (c_proj) interleaved with the final drains ----
            def emit_e(ic):
                ot = outp.tile([128, D], f32, name="ot")
                for n2 in range(2):
                    pc = psD.tile([128, 384], f32, name="pc", tag="pdc")
                    for c in range(3):
                        nc.tensor.matmul(pc[:], hoT[c][:, 128 * ic:128 * (ic + 1)],
                                         cws[c][:, 384 * n2:384 * (n2 + 1)],
                                         start=(c == 0), stop=(c == 2))
                    copy_out(2 * ic + n2, ot[:, 384 * n2:384 * (n2 + 1)], pc[:])
                nc.sync.dma_start(out=out[128 * ic:128 * (ic + 1), :], in_=ot[:])

            emit_d_group(pending)          # (h4, half 0)
            emit_d_group(pending)          # (h5, half 0) -> half 0 complete
            emit_e(0)
            emit_d_group(pending)          # (h4, half 1)
            emit_e(1)
            emit_d_group(pending)          # (h5, half 1)
            for ic in range(2, 8):
                emit_e(ic)
        ab.close()  # frees x/W/QTaug/TKaug sbuf

    nc.compile()
    return nc


_NC_CACHE = None
_NC_KEY = None


def _get_nc(with_bias=True):
    global _NC_CACHE, _NC_KEY
    if _NC_CACHE is None or _NC_KEY != with_bias:
        _NC_CACHE = build_bass(with_bias=with_bias)
        _NC_KEY = with_bias
    return _NC_CACHE


def make_in_maps(x, attention_mask, Wc_w, Wc_b, Wp_w, table, cproj_w):
    x = np.asarray(x, np.float32)
    attention_mask = np.asarray(attention_mask)
    Wc_w = np.asarray(Wc_w, np.float32)
    Wc_b = np.asarray(Wc_b, np.float32)
    Wp_w = np.asarray(Wp_w, np.float32)
    table = np.asarray(table, np.float32)
    cproj_w = np.asarray(cproj_w, np.float32)

    scale = 1.0 / math.sqrt(DH)
    idx = np.clip(639 - np.arange(W_CPE), 0, 512)
    tpad_np = np.ascontiguousarray(table.T[:, idx])
    wpq_np = np.ascontiguousarray(Wp_w[:, 0:DH]) * scale
    wpk_np = np.ascontiguousarray(Wp_w[:, DH:2 * DH])

    in_maps = []
    for c in range(NCORES):
        b, hg = divmod(c, 2)
        sl = slice(hg * DG, (hg + 1) * DG)
        bf = ml_dtypes.bfloat16
        xt_c = np.ascontiguousarray(x[b].T)
        in_maps.append({
            "xtb": xt_c.astype(bf),
            "wq": (np.ascontiguousarray(Wc_w[:, sl]) * scale).astype(bf),
            "wk": np.ascontiguousarray(Wc_w[:, D + hg * DG: D + (hg + 1) * DG]).astype(bf),
            "wv": np.ascontiguousarray(Wc_w[:, 2 * D + hg * DG: 2 * D + (hg + 1) * DG]).astype(bf),
            "bq": (Wc_b[sl] * scale).reshape(1, DG).astype(bf),
            "bk": Wc_b[D + hg * DG: D + (hg + 1) * DG].reshape(1, DG).astype(bf),
            "bv": Wc_b[2 * D + hg * DG: 2 * D + (hg + 1) * DG].reshape(1, DG).astype(bf),
            "cw": np.ascontiguousarray(cproj_w[sl, :]).astype(bf),
            "tpad": tpad_np.astype(bf),
            "wpq": wpq_np.astype(bf),
            "wpk": wpk_np.astype(bf),
            "mb": np.where(attention_mask[b] == 0, -1e9, 0.0).astype(np.float32),
        })
    return in_maps


def kernel(x, attention_mask, Wc_w, Wc_b, Wp_w, table, cproj_w, cproj_b,
           n_h, k, **_ignored):
    global LAST_RESULT
    assert int(n_h) == NH and int(k) == KC
    in_maps = make_in_maps(x, attention_mask, Wc_w, Wc_b, Wp_w, table, cproj_w)
    wb = bool(np.any(np.asarray(Wc_b) != 0))
    nc = _get_nc(with_bias=wb)
    res = run_bass_kernel_spmd(nc, in_maps, list(range(NCORES)))
    LAST_RESULT = res
    outs = res.results
    full = np.zeros((B, S, D), np.float32)
    for b in range(B):
        full[b] = outs[2 * b]["out"] + outs[2 * b + 1]["out"]
    full += np.asarray(cproj_b, np.float32)[None, None, :]
    return full


# revision 21
# speedup vs baseline: 1.4092x; 1.0831x over previous
"""Trainium2 Bass kernel: disentangled (DeBERTa-style) attention.

Full inputs in, full output out. Sharding: data-parallel over batch (4) x
tensor-parallel over head-groups (2) = 8 cores. Core c handles batch c//2,
heads (c%2)*6 .. +6. The relative-position tensors are replicated.

Key algebraic facts exploited:
  * P = table[rel] has only 513 distinct rows (rel depends on j-i only), so
    qr/kr = (P @ Wp) collapses to tableW = table @ Wp plus an index map.
  * c_p[i,s] = qc[i] . tableK[r],  c_r[i,s] = (tableQ[r] . kc_sum), with
    r = clip(i-s+256, 0, 512).  Both fold into one per-head strip
    CPc[i, r] = qc[i] . tableK[r] + cr[r]  of shape [S, 513].
  * The score contribution extra[i,s] = CPc[i, r(i,s)] is a Toeplitz skew of
    that strip: materialize a padded, reversed strip CPcE[i, u] (u in [0,768))
    in DRAM, then read 128x640 parallelogram tiles with a DMA access pattern
    whose partition stride is (768-1) elements -- each SBUF partition gets a
    contiguous run, so the DMA runs at line rate.  PE transpose-matmuls
    accumulate those tiles into the transposed score PSUM.  Fully saturated
    blocks (|i-s| > 383) are rank-1 and handled by K=65 matmuls whose
    stationary operand has satcol broadcast to every column.
  * Strip generation for pair p+1 is software-pipelined into the score loop
    of pair p (and pair 0's into phase A) so the PE instruction stream stays
    dense -- TRN2's HAM clock gate halves the PE clock whenever the stream
    goes idle-ish, which was the dominant cost of the phase-serial version.

Score layout is transposed ([s on partitions, i free]) so that attn@V needs
no transposes: out_raw[i,d] = sum_s exp[s,i] V[s,d] contracts s on the
partition dim, the softmax denominator rides along as a ones-column of V,
and normalization is a per-partition tensor_scalar.
"""

import math
from contextlib import ExitStack

import ml_dtypes
import numpy as np

import concourse.bass as bass
from concourse import bacc
import concourse.mybir as mybir
import concourse.tile as tile
from concourse.bass_utils import run_bass_kernel_spmd
from concourse.masks import make_identity

f32 = mybir.dt.float32
bf16 = mybir.dt.bfloat16

B, S, D = 4, 1024, 768
NH, DH, KC = 12, 64, 256
HPC = NH // 2          # heads per core = 6
DG = HPC * DH          # 384 head-dims per core
W_CPE = 768            # padded skew strip width (127 | 513 | 128)
NCORES = 8

LAST_RESULT = None     # BassKernelResults of the most recent run (for tests)


def _sat_ranges(J):
    """Fully saturated column ranges of transposed-score block-row J.

    Returns (sat_row, lo, hi) triples: sat_row 0 => r=512 (i-s >= 257),
    sat_row 1 => r=0 (i-s <= -257).  Ranges are split at the 512-column PSUM
    bank boundary.
    """
    out = []
    lo = 128 * (J + 3)           # i >= 128*(J+3)  -> r = 512
    if lo < S:
        for b0 in (0, 512):
            a, b = max(lo, b0), min(S, b0 + 512)
            if a < b:
                out.append((0, a, b))
    hi = 128 * (J - 2)           # i < 128*(J-2)   -> r = 0
    if hi > 0:
        for b0 in (0, 512):
            a, b = max(0, b0), min(hi, b0 + 512)
            if a < b:
                out.append((1, a, b))
    return out


def build_bass(with_bias=True):
    nc = bacc.Bacc("TRN2", target_bir_lowering=False)

    xtb = nc.dram_tensor("xtb", [D, S], bf16, kind="ExternalInput")
    wq = nc.dram_tensor("wq", [D, DG], bf16, kind="ExternalInput")
    wk = nc.dram_tensor("wk", [D, DG], bf16, kind="ExternalInput")
    wv = nc.dram_tensor("wv", [D, DG], bf16, kind="ExternalInput")
    bq = nc.dram_tensor("bq", [1, DG], bf16, kind="ExternalInput")
    bk = nc.dram_tensor("bk", [1, DG], bf16, kind="ExternalInput")
    bv = nc.dram_tensor("bv", [1, DG], bf16, kind="ExternalInput")
    cw = nc.dram_tensor("cw", [DG, D], bf16, kind="ExternalInput")
    tpad = nc.dram_tensor("tpad", [DH, W_CPE], bf16, kind="ExternalInput")
    wpq = nc.dram_tensor("wpq", [DH, DH], bf16, kind="ExternalInput")
    wpk = nc.dram_tensor("wpk", [DH, DH], bf16, kind="ExternalInput")
    mb = nc.dram_tensor("mb", [S], f32, kind="ExternalInput")
    out = nc.dram_tensor("out", [S, D], f32, kind="ExternalOutput")
    # per-pair skew strip scratch (separate tensors so pair p+1's writes
    # never serialize against pair p's reads)
    cpe = [nc.dram_tensor(f"cpe{t}", [2, S, W_CPE], bf16) for t in range(3)]

    with tile.TileContext(nc) as tc, ExitStack() as ex:
        const = ex.enter_context(tc.tile_pool(name="const", bufs=1))
        persist = ex.enter_context(tc.tile_pool(name="persist", bufs=1))
        # strip-generation pools live from phase A through the score loop
        psB = ex.enter_context(tc.tile_pool(name="psB", space="PSUM", bufs=2))
        cpool = ex.enter_context(tc.tile_pool(name="cpool", bufs=6))

        ident_b = const.tile([128, 128], bf16, name="ident_b")
        make_identity(nc, ident_b[:])
        ones_f = const.tile([1, 512], bf16, name="ones_f")
        nc.vector.memset(ones_f[:], 1.0)
        ones65 = const.tile([65, 128], bf16, name="ones65")
        nc.vector.memset(ones65[:], 1.0)
        mb_sb = const.tile([128, 8], f32, name="mb_sb")

        QT = [persist.tile([128, S], bf16, name=f"QT{t}") for t in range(3)]
        KT = [persist.tile([128, S], bf16, name=f"KT{t}") for t in range(3)]
        Vb = [persist.tile([128, HPC * 128], bf16, name=f"Vb{j}") for j in range(8)]
        cws = [persist.tile([128, D], bf16, name=f"cw{c}") for c in range(3)]
        TQp = persist.tile([DH, W_CPE], bf16, name="TQp")
        kcs = [persist.tile([128, 1], f32, name=f"kcs{t}") for t in range(3)]
        # satb[h][p]: [65,128] stationary whose every column = satcols[h][:,p]
        satb = [[persist.tile([65, 128], bf16, name=f"satb{h}_{p}") for p in range(2)]
                for h in range(HPC)]

        def copy_out(k, dst, src):
            """PSUM->SBUF drain; GPSIMD has no PSUM port, so alternate the
            two engines that do."""
            if k % 2 == 1:
                nc.scalar.copy(dst, src)
            else:
                nc.vector.tensor_copy(dst, src)

        # ---------------- Phase A: QKV projection + tables ----------------
        ab = ExitStack()
        wld = ExitStack()
        abp = ab.enter_context(tc.tile_pool(name="abp", bufs=1))
        wload = wld.enter_context(tc.tile_pool(name="wload", bufs=1))

        xbs, wqs, wks, wvs = [], [], [], []
        for t in range(6):
            w = wload.tile([128, DG], bf16, name=f"wq{t}")
            nc.sync.dma_start(out=w[:], in_=wq[128 * t:128 * (t + 1), :])
            wqs.append(w)
            xb = wload.tile([128, S], bf16, name=f"xbt{t}")
            nc.sync.dma_start(out=xb[:], in_=xtb[128 * t:128 * (t + 1), :])
            xbs.append(xb)
        for nm, dram, lst in (("wk", wk, wks), ("wv", wv, wvs)):
            for t in range(6):
                w = wload.tile([128, DG], bf16, name=f"{nm}{t}")
                nc.sync.dma_start(out=w[:], in_=dram[128 * t:128 * (t + 1), :])
                lst.append(w)
        for c in range(3):
            nc.sync.dma_start(out=cws[c][:], in_=cw[128 * c:128 * (c + 1), :])
        bq_sb = wload.tile([1, DG], bf16, name="bq_sb")
        bk_sb = wload.tile([1, DG], bf16, name="bk_sb")
        bv_sb = wload.tile([1, DG], bf16, name="bv_sb")
        nc.sync.dma_start(out=bq_sb[:], in_=bq[:, :])
        nc.sync.dma_start(out=bk_sb[:], in_=bk[:, :])
        nc.sync.dma_start(out=bv_sb[:], in_=bv[:, :])
        nc.sync.dma_start(out=mb_sb[:], in_=bass.AP(mb, 0, [[1, 128], [128, 8]]))
        tpad_sb = wload.tile([DH, W_CPE], bf16, name="tpad_sb")
        nc.sync.dma_start(out=tpad_sb[:], in_=tpad[:, :])
        wpq_sb = wload.tile([DH, DH], bf16, name="wpq_sb")
        nc.sync.dma_start(out=wpq_sb[:], in_=wpq[:, :])
        wpk_sb = wload.tile([DH, DH], bf16, name="wpk_sb")
        nc.sync.dma_start(out=wpk_sb[:], in_=wpk[:, :])

        # per-head augmented operand tiles (all pairs; alive through scores)
        QTaug = [abp.tile([65, S], bf16, name=f"QTaug{h}") for h in range(HPC)]
        TKaug = [abp.tile([65, W_CPE], bf16, name=f"TKaug{h}") for h in range(HPC)]
        kc_col = [abp.tile([DH, 1], bf16, name=f"kc{h}") for h in range(HPC)]
        satcols = [abp.tile([65, 2], f32, name=f"satc{h}") for h in range(HPC)]
        crh_sb = [abp.tile([1, W_CPE], bf16, name=f"crh{h}") for h in range(HPC)]
        TKp_sb = wload.tile([DH, W_CPE], bf16, name="TKp_sb")

        strip_ct = {}        # (h, I) -> SBUF tile pending DMA (none kept)

        def emit_strip(h, I):
            """One 128-row block of head h's padded skew strip -> DRAM."""
            t, r = divmod(h, 2)
            lhs = QTaug[h][:, 128 * I:128 * (I + 1)]
            ct = cpool.tile([128, W_CPE], bf16, name="cpe_t")
            for k, (w0, w1) in enumerate(((0, 512), (512, W_CPE))):
                pab = psB.tile([128, w1 - w0], f32, name="psB_t", tag="psB")
                nc.tensor.matmul(pab[:], lhs, TKaug[h][:, w0:w1],
                                 start=True, stop=True)
                copy_out(2 * (8 * h + I) + k, ct[:, w0:w1], pab[:])
            nc.sync.dma_start(out=cpe[t][r, 128 * I:128 * (I + 1), :], in_=ct[:])

        def emit_qkt(dst, wlist, brow, m, n2):
            ps = psA.tile([128, 512], f32, name="psA_t", tag="psA")
            for kk in range(6):
                nc.tensor.matmul(
                    ps[:], wlist[kk][:, 128 * m:128 * (m + 1)],
                    xbs[kk][:, 512 * n2:512 * (n2 + 1)],
                    start=(kk == 0), stop=(kk == 5 and not with_bias))
            if with_bias:
                nc.tensor.matmul(
                    ps[:], brow[0:1, 128 * m:128 * (m + 1)],
                    ones_f[0:1, :], start=False, stop=True)
            copy_out(m + n2, dst[m][:, 512 * n2:512 * (n2 + 1)], ps[:])

        def emit_v(j):
            ps = psA.tile([128, DG], f32, name="psA_v", tag="psA")
            for kk in range(6):
                nc.tensor.matmul(
                    ps[:], xbs[kk][:, 128 * j:128 * (j + 1)], wvs[kk][:],
                    start=(kk == 0), stop=(kk == 5 and not with_bias))
            if with_bias:
                nc.tensor.matmul(ps[:], ones_f[0:1, 0:128], bv_sb[0:1, :],
                                 start=False, stop=True)
            vdst = Vb[j][:].rearrange("p (h c) -> p h c", h=HPC)
            nc.vector.memset(vdst[:, :, 64:128], 0.0)
            nc.vector.tensor_copy(vdst[:, :, 0:64],
                                  ps[:].rearrange("p (h c) -> p h c", h=HPC))
            nc.vector.memset(vdst[:, :, 64:65], 1.0)

        def emit_pair_setup(t):
            """kc_sum, QTaug/TKaug/crh/satcols/satb for pair t's two heads."""
            nc.vector.tensor_reduce(kcs[t][:], KT[t][:],
                                    axis=mybir.AxisListType.X,
                                    op=mybir.AluOpType.add)
            for r in range(2):
                h = 2 * t + r
                nc.sync.dma_start(out=QTaug[h][0:64, :],
                                  in_=QT[t][64 * r:64 * r + 64, :])
                nc.vector.memset(QTaug[h][64:65, :], 1.0)
                nc.vector.tensor_copy(TKaug[h][0:64, :], TKp_sb[:])
                nc.gpsimd.dma_start(out=kc_col[h][:],
                                    in_=kcs[t][64 * r:64 * r + 64, 0:1])
            for r in range(2):
                h = 2 * t + r
                ps = psT.tile([1, W_CPE], f32, name="psT_cr", tag="psT")
                nc.tensor.matmul(ps[:, 0:512], kc_col[h][:], TQp[:, 0:512],
                                 start=True, stop=True)
                nc.tensor.matmul(ps[:, 512:W_CPE], kc_col[h][:], TQp[:, 512:W_CPE],
                                 start=True, stop=True)
                nc.vector.tensor_copy(crh_sb[h][:], ps[:])
                nc.sync.dma_start(out=TKaug[h][64:65, :], in_=crh_sb[h][:])
                nc.vector.tensor_copy(satcols[h][:, 0:1], TKaug[h][:, 127:128])
                nc.vector.tensor_copy(satcols[h][:, 1:2], TKaug[h][:, 639:640])
                for p in range(2):
                    nc.vector.tensor_scalar(
                        out=satb[h][p][:], in0=ones65[:],
                        scalar1=satcols[h][:, p:p + 1], scalar2=None,
                        op0=mybir.AluOpType.mult)

        with tc.tile_pool(name="psA", space="PSUM", bufs=2) as psA, \
             tc.tile_pool(name="psT", space="PSUM", bufs=2) as psT:
            # pair 0's Q^T/K^T first so its strip pipeline can start early
            for m, n2 in ((0, 0), (0, 1)):
                emit_qkt(QT, wqs, bq_sb, m, n2)
            for m, n2 in ((0, 0), (0, 1)):
                emit_qkt(KT, wks, bk_sb, m, n2)
            # tableW strips: TKp = Wp_k^T @ tpad, TQp = Wp_q^T @ tpad
            for wsb, dsts in ((wpk_sb, TKp_sb), (wpq_sb, TQp)):
                ps = psT.tile([DH, W_CPE], f32, name="psT_t", tag="psT")
                nc.tensor.matmul(ps[:, 0:512], wsb[:], tpad_sb[:, 0:512],
                                 start=True, stop=True)
                nc.tensor.matmul(ps[:, 512:W_CPE], wsb[:], tpad_sb[:, 512:W_CPE],
                                 start=True, stop=True)
                nc.vector.tensor_copy(dsts[:], ps[:])
            emit_pair_setup(0)

            # interleave pair 0's 16 strip blocks with the rest of phase A
            rest = [("qkt", QT, wqs, bq_sb, 1, 0), ("qkt", QT, wqs, bq_sb, 1, 1),
                    ("qkt", KT, wks, bk_sb, 1, 0), ("qkt", KT, wks, bk_sb, 1, 1),
                    ("setup", 1),
                    ("qkt", QT, wqs, bq_sb, 2, 0), ("qkt", QT, wqs, bq_sb, 2, 1),
                    ("qkt", KT, wks, bk_sb, 2, 0), ("qkt", KT, wks, bk_sb, 2, 1),
                    ("setup", 2),
                    ("v", 0), ("v", 1), ("v", 2), ("v", 3),
                    ("v", 4), ("v", 5), ("v", 6), ("v", 7)]
            strips0 = [(h, I) for I in range(8) for h in (0, 1)]
            ri = si = 0
            while ri < len(rest) or si < len(strips0):
                if ri < len(rest):
                    op_ = rest[ri]; ri += 1
                    if op_[0] == "qkt":
                        emit_qkt(*op_[1:])
                    elif op_[0] == "v":
                        emit_v(op_[1])
                    else:
                        emit_pair_setup(op_[1])
                if si < len(strips0):
                    h, I = strips0[si]; si += 1
                    emit_strip(h, I)
        wld.close()  # x/weight staging dead after phase A

        # ----- score/attn loop per head-pair, strips(p+1) pipelined in -----
        hoT = [persist.tile([128, S], bf16, name=f"hoT{c}") for c in range(3)]
        with tc.tile_pool(name="psC", space="PSUM", bufs=2) as psC, \
             tc.tile_pool(name="psD", space="PSUM", bufs=2) as psD, \
             tc.tile_pool(name="srp", bufs=18) as srp, \
             tc.tile_pool(name="expp", bufs=22) as expp, \
             tc.tile_pool(name="rcp", bufs=6) as rcp, \
             tc.tile_pool(name="rbp", bufs=2) as rbp, \
             tc.tile_pool(name="outp", bufs=4) as outp, \
             tc.tile_pool(name="otp", bufs=2) as otp:

            def emit_d_group(dstate):
                """Emit one attn@V (h, half) group of the previous pair."""
                h, half, exps = dstate.pop(0)
                t, r = divmod(h, 2)
                pd = psD.tile([128, 512], f32, name="pd", tag="pdc")
                for J in range(8):
                    nc.tensor.matmul(
                        pd[:], Vb[J][:, 128 * h:128 * (h + 1)],
                        exps[(h, J)][:, 512 * half:512 * (half + 1)],
                        start=(J == 0), stop=(J == 7))
                rc0 = rcp.tile([1, 512], f32, name="rc0")
                nc.scalar.copy(rc0[:], pd[64:65, :])
                rc = rcp.tile([1, 512], f32, name="rc")
                nc.vector.reciprocal_approx_fast(rc[:], rc0[:])
                rb = rbp.tile([64, 512], f32, name="rb")
                nc.gpsimd.partition_broadcast(rb[:], rc[:])
                if not r:
                    dstv = hoT[t][0:64, 512 * half:512 * (half + 1)]
                else:
                    tmpo = dstate_tmp.setdefault(h, otp.tile([64, S], bf16, name="tmpo"))
                    dstv = tmpo[:, 512 * half:512 * (half + 1)]
                nc.vector.tensor_tensor(out=dstv, in0=pd[0:64, :], in1=rb[:],
                                        op=mybir.AluOpType.mult)
                if r:
                    tm = dstate_tmp[h]
                    nc.sync.dma_start(out=hoT[t][64:128, 512 * half:512 * (half + 1)],
                                      in_=tm[:, 512 * half:512 * (half + 1)])
                    if half == 1:
                        dstate_tmp.pop(h)

            pending = []          # (h, half, expT-dict) groups awaiting attn@V
            dstate_tmp = {}
            prefetched = {}       # (h, I) -> sr tile loaded ahead of its pair

            def load_sr(h, I):
                t, r = divmod(h, 2)
                sr = srp.tile([128, 640], bf16, name="sr")
                s_lo = max(0, 128 * (I - 2))
                s_hi = min(S, 128 * (I + 3))
                s0 = s_lo - 128 * (I - 2)
                Wd = s_hi - s_lo
                off = (r * S + 128 * I) * W_CPE + 127 + s0
                srcap = bass.AP(cpe[t], off, [[W_CPE - 1, 128], [1, Wd]])
                nc.sync.dma_start(out=sr[:, s0:s0 + Wd], in_=srcap)
                return sr

            for hp in range(3):
                heads = (2 * hp, 2 * hp + 1)
                SRs = {h: {} for h in heads}
                expT = {}
                for J in range(8):
                    # next pair's strip blocks, two per step
                    if hp < 2:
                        for h2 in (2 * hp + 2, 2 * hp + 3):
                            emit_strip(h2, J)
                    # prefetch distance 2: tile I is issued at step I-4 so the
                    # ~2.5us DMA (latency + 160KB) never gates the transposes
                    new_Is = range(0, 5) if J == 0 else \
                        (range(J + 4, J + 5) if J + 4 < 8 else range(0))
                    for h in heads:
                        for I in new_Is:
                            if (h, I) in prefetched:
                                SRs[h][I] = prefetched.pop((h, I))
                            else:
                                SRs[h][I] = load_sr(h, I)
                    if hp < 2 and 4 <= J <= 7:
                        # warm up next pair's J=0 window (2 tiles per step)
                        I2 = J - 4
                        for h2 in (2 * hp + 2, 2 * hp + 3):
                            prefetched[(h2, I2)] = load_sr(h2, I2)
                    scs = {}
                    for h in heads:
                        bank_ops = {0: [("cc", 0)], 1: [("cc", 1)]}
                        for I in range(max(0, J - 2), min(8, J + 3)):
                            bank_ops[I // 4].append(("tr", I))
                        for row, lo, hi in _sat_ranges(J):
                            bank_ops[lo // 512].append(("sat", (row, lo, hi)))
                        last = {b: ops[-1] for b, ops in bank_ops.items()}
                        sc = psC.tile([128, S], f32, name="sc")
                        scs[h] = (sc, [sc[:, 0:512], sc[:, 512:S]], last)
                    for n2 in range(2):
                        for h in heads:
                            t, r = divmod(h, 2)
                            sc, halves, last = scs[h]
                            nc.tensor.matmul(
                                halves[n2],
                                KT[t][64 * r:64 * r + 64, 128 * J:128 * (J + 1)],
                                QT[t][64 * r:64 * r + 64, 512 * n2:512 * (n2 + 1)],
                                start=True, stop=(last[n2] == ("cc", n2)),
                                tile_position=(64 * r, 0) if r else None)
                    for h in heads:
                        sc, halves, last = scs[h]
                        for I in range(max(0, J - 2), min(8, J + 3)):
                            dlt = I - J
                            nc.tensor.matmul(
                                halves[I // 4][:, 128 * (I % 4):128 * (I % 4 + 1)],
                                SRs[h][I][:, 128 * (2 - dlt):128 * (3 - dlt)],
                                ident_b[:],
                                start=False,
                                stop=(last[I // 4] == ("tr", I)))
                        for row, lo, hi in _sat_ranges(J):
                            b = lo // 512
                            nc.tensor.matmul(
                                halves[b][:, lo - 512 * b:hi - 512 * b],
                                satb[h][row][:], QTaug[h][:, lo:hi],
                                start=False,
                                stop=(last[b] == ("sat", (row, lo, hi))))
                        et = expp.tile([128, S], bf16, name="et")
                        nc.scalar.activation(et[:], sc[:],
                                             mybir.ActivationFunctionType.Exp,
                                             bias=mb_sb[:, J:J + 1], scale=1.0)
                        expT[(h, J)] = et
                    # interleave: drain one previous-pair attn@V group per
                    # even J -- spreading the recip/normalize DVE cost keeps
                    # the PE stream dense at pair boundaries (HAM warmth)
                    if J % 2 == 0 and pending:
                        emit_d_group(pending)
                for h in heads:
                    for half in range(2):
                        pending.append((h, half, expT))
            pending.sort(key=lambda g: g[1])

            # ---- Phase E (c_proj) interleaved with the final drains ----
            def emit_e(ic):
                ot = outp.tile([128, D], f32, name="ot")
                for n2 in range(2):
                    pc = psD.tile([128, 384], f32, name="pc", tag="pdc")
                    for c in range(3):
                        nc.tensor.matmul(pc[:], hoT[c][:, 128 * ic:128 * (ic + 1)],
                                         cws[c][:, 384 * n2:384 * (n2 + 1)],
                                         start=(c == 0), stop=(c == 2))
                    copy_out(2 * ic + n2, ot[:, 384 * n2:384 * (n2 + 1)], pc[:])
                nc.sync.dma_start(out=out[128 * ic:128 * (ic + 1), :], in_=ot[:])

            emit_d_group(pending)          # (h4, half 0)
            emit_d_group(pending)          # (h5, half 0) -> half 0 complete
            emit_e(0)
            emit_d_group(pending)          # (h4, half 1)
            emit_e(1)
            emit_d_group(pending)          # (h5, half 1)
            for ic in range(2, 8):
                emit_e(ic)
        ab.close()  # frees x/W/QTaug/TKaug sbuf

    nc.compile()
    return nc


_NC_CACHE = None
_NC_KEY = None


def _get_nc(with_bias=True):
    global _NC_CACHE, _NC_KEY
    if _NC_CACHE is None or _NC_KEY != with_bias:
        _NC_CACHE = build_bass(with_bias=with_bias)
        _NC_KEY = with_bias
    return _NC_CACHE


def make_in_maps(x, attention_mask, Wc_w, Wc_b, Wp_w, table, cproj_w):
    x = np.asarray(x, np.float32)
    attention_mask = np.asarray(attention_mask)
    Wc_w = np.asarray(Wc_w, np.float32)
    Wc_b = np.asarray(Wc_b, np.float32)
    Wp_w = np.asarray(Wp_w, np.float32)
    table = np.asarray(table, np.float32)
    cproj_w = np.asarray(cproj_w, np.float32)

    scale = 1.0 / math.sqrt(DH)
    idx = np.clip(639 - np.arange(W_CPE), 0, 512)
    tpad_np = np.ascontiguousarray(table.T[:, idx])
    wpq_np = np.ascontiguousarray(Wp_w[:, 0:DH]) * scale
    wpk_np = np.ascontiguousarray(Wp_w[:, DH:2 * DH])

    in_maps = []
    for c in range(NCORES):
        b, hg = divmod(c, 2)
        sl = slice(hg * DG, (hg + 1) * DG)
        bf = ml_dtypes.bfloat16
        xt_c = np.ascontiguousarray(x[b].T)
        in_maps.append({
            "xtb": xt_c.astype(bf),
            "wq": (np.ascontiguousarray(Wc_w[:, sl]) * scale).astype(bf),
            "wk": np.ascontiguousarray(Wc_w[:, D + hg * DG: D + (hg + 1) * DG]).astype(bf),
            "wv": np.ascontiguousarray(Wc_w[:, 2 * D + hg * DG: 2 * D + (hg + 1) * DG]).astype(bf),
            "bq": (Wc_b[sl] * scale).reshape(1, DG).astype(bf),
            "bk": Wc_b[D + hg * DG: D + (hg + 1) * DG].reshape(1, DG).astype(bf),
            "bv": Wc_b[2 * D + hg * DG: 2 * D + (hg + 1) * DG].reshape(1, DG).astype(bf),
            "cw": np.ascontiguousarray(cproj_w[sl, :]).astype(bf),
            "tpad": tpad_np.astype(bf),
            "wpq": wpq_np.astype(bf),
            "wpk": wpk_np.astype(bf),
            "mb": np.where(attention_mask[b] == 0, -1e9, 0.0).astype(np.float32),
        })
    return in_maps


def kernel(x, attention_mask, Wc_w, Wc_b, Wp_w, table, cproj_w, cproj_b,
           n_h, k, **_ignored):
    global LAST_RESULT
    assert int(n_h) == NH and int(k) == KC
    in_maps = make_in_maps(x, attention_mask, Wc_w, Wc_b, Wp_w, table, cproj_w)
    wb = bool(np.any(np.asarray(Wc_b) != 0))
    nc = _get_nc(with_bias=wb)
    res = run_bass_kernel_spmd(nc, in_maps, list(range(NCORES)))
    LAST_RESULT = res
    outs = res.results
    full = np.zeros((B, S, D), np.float32)
    for b in range(B):
        full[b] = outs[2 * b]["out"] + outs[2 * b + 1]["out"]
    full += np.asarray(cproj_b, np.float32)[None, None, :]
    return full
